# revision 4
# baseline (speedup 1.0000x reference)
"""Causal GQA self-attention block (B=4, S=2048, D=2048, 16 q-heads / 4 kv-heads)
on 8 Trainium2 NeuronCores.

Sharding: TP2 x DP4. Core c handles batch b = c//2 and head-half h = c%2
(q-heads 8h..8h+7, kv-heads 2h..2h+1). Each core computes a [2048, 2048]
partial of the output projection (transposed, [out_dim, seq]); the host sums
the two TP partials per batch and transposes back.

Per-core pipeline (matmuls bf16 inputs / fp32 PSUM accumulation):
  A. QKV projection per 128-token tile; PSUM evicted once to SBUF bf16;
     RMS statistics on ACT (Square + accum, Rsqrt); RoPE on DVE in bf16
     2x mode against host-pre-tiled cos/sin tables; per-head gain scaling;
     PE-transpose of Q,K head tiles into persistent [hd, seq] buffers
     (V kept natural [seq, hd]).
  B. Attention per head in transposed layout: S^T[k,q] blocks = K_blk^T Q^T
     with causally-trimmed query ranges for diagonal-band blocks, plain exp
     (rms-normed q,k bound |score| <= sqrt(hd)), gpsimd affine_select zeroes
     the strict upper triangle of diagonal 128x128 chunks, softmax sums
     accumulated with gpsimd pair-adds + DVE f32 adds, denominator via a
     single ones-matmul partition reduction + reciprocal_approx_fast + PE
     broadcast, unnormalized y^T accumulated in PSUM and normalized at
     eviction.
  C. Output projection: out^T[o, s] accumulated over the core's 8 heads.
"""
import sys

if "/opt/trn_rl_repo" not in sys.path:
    sys.path.insert(0, "/opt/trn_rl_repo")

import numpy as np
import ml_dtypes

import concourse.bass as bass
import concourse.mybir as mybir
from concourse import bacc
from concourse.tile import TileContext
from concourse.bass_utils import run_bass_kernel_spmd
from concourse.masks import make_identity

BF16 = mybir.dt.bfloat16
F32 = mybir.dt.float32
AF = mybir.ActivationFunctionType
OP = mybir.AluOpType

DIM = 2048
SEQ = 2048
BATCH = 4
HD = 128
NH_L = 8            # q heads per core
NKV_L = 2           # kv heads per core
NH_T = NH_L + NKV_L  # normed/roped heads per tile
QKV = (NH_L + 2 * NKV_L) * HD   # 1536
N_ST = SEQ // 128   # 16 seq tiles
N_DC = DIM // 128   # 16 contraction chunks
N_QT = SEQ // 512   # 4 query tiles of 512
EPS = 1.1920928955078125e-07
N_CORES = 8

_CACHED_NC = None


def _build_nc():
    nc = bacc.Bacc(
        "TRN2",
        target_bir_lowering=False,
        debug=False,
        num_devices=N_CORES,
    )
    xT = nc.dram_tensor("xT", [DIM, SEQ], BF16, kind="ExternalInput")
    wt = nc.dram_tensor("wt", [DIM, QKV], BF16, kind="ExternalInput")
    wpT = nc.dram_tensor("wpT", [NH_L * HD, DIM], BF16, kind="ExternalInput")
    csrep = nc.dram_tensor("csrep", [SEQ, 2 * NH_T * 64], BF16,
                           kind="ExternalInput")
    gains = nc.dram_tensor("gains", [128, NH_T], F32, kind="ExternalInput")
    out = nc.dram_tensor("out", [DIM, SEQ], F32, kind="ExternalOutput")

    with TileContext(nc) as tc, \
         nc.allow_low_precision(reason="f32r matmuls for softmax denominators"):
        with tc.tile_pool(name="const", bufs=1) as const, \
             tc.tile_pool(name="persist", bufs=1) as persist:
            ident = const.tile([128, 128], BF16)
            make_identity(nc, ident[:])
            ones_col = const.tile([128, 1], F32)
            nc.gpsimd.memset(ones_col[:], 1.0)
            ones_row = const.tile([1, 128], F32)
            nc.gpsimd.memset(ones_row[:], 1.0)
            eps_sb = const.tile([128, 1], F32)
            nc.gpsimd.memset(eps_sb[:], EPS)
            gains_sb = const.tile([128, NH_T], F32)
            nc.sync.dma_start(gains_sb[:], gains[:])
            # QKV weights: one tile per 128-row contraction chunk so the
            # first matmul only waits on its own chunk's DMA
            wt_sb = []
            for c in range(N_DC):
                t = const.tile([128, QKV], BF16, tag=f"wt{c}")
                nc.sync.dma_start(t[:], wt[c * 128:(c + 1) * 128, :])
                wt_sb.append(t)
            # proj weights: 8 tiles [128 (head-dim slice), DIM]
            wp_sb = []
            for ic in range(NH_L):
                t = const.tile([128, DIM], BF16, tag=f"wp{ic}")
                nc.sync.dma_start(t[:], wpT[ic * 128:(ic + 1) * 128, :])
                wp_sb.append(t)

            # persistent activations
            qt_sb = persist.tile([128, NH_L * SEQ], BF16)   # Q^T per head
            kt_sb = persist.tile([128, NKV_L * SEQ], BF16)  # K^T per kv head
            v_sb = persist.tile([128, N_ST * NKV_L * HD], BF16)  # V natural
            yt_sb = persist.tile([128, NH_L * SEQ], BF16)   # y^T per head

            # ---------------- stage A: QKV projection + norm/rope/transpose
            with tc.tile_pool(name="a_sbuf", bufs=2) as a_sbuf, \
                 tc.tile_pool(name="a_stat", bufs=2) as a_stat, \
                 tc.tile_pool(name="a_psum", bufs=2, space="PSUM") as a_psum, \
                 tc.tile_pool(name="t_psum", bufs=1, space="PSUM") as t_psum:
                xt_pair = None
                for st in range(N_ST):
                    if st % 2 == 0:
                        # load xT for two seq-tiles at once: 512B contiguous
                        # runs per partition
                        xt_pair = a_sbuf.tile([128, N_DC * 256], BF16, tag="xt")
                        for c in range(N_DC):
                            nc.sync.dma_start(
                                xt_pair[:, c * 256:(c + 1) * 256],
                                xT[c * 128:(c + 1) * 128,
                                   st * 128:(st + 2) * 128])
                    half = st % 2
                    cs_t = a_sbuf.tile([128, 2 * NH_T * 64], BF16, tag="cs")
                    nc.sync.dma_start(cs_t[:], csrep[st * 128:(st + 1) * 128, :])

                    qkv_ps = a_psum.tile([128, QKV], F32, tag="qkv")
                    for c in range(N_DC):
                        for n in range(QKV // 512):
                            nc.tensor.matmul(
                                qkv_ps[:, n * 512:(n + 1) * 512],
                                xt_pair[:, c * 256 + half * 128:
                                        c * 256 + half * 128 + 128],
                                wt_sb[c][:, n * 512:(n + 1) * 512],
                                start=(c == 0), stop=(c == N_DC - 1))

                    # rms statistics for the 10 normed heads (ACT)
                    sq = a_stat.tile([128, 128], F32, tag="sq")
                    ssq = a_stat.tile([128, NH_T], F32, tag="ssq")
                    for j in range(NH_T):
                        nc.scalar.activation(
                            sq[:], qkv_ps[:, j * 128:(j + 1) * 128],
                            AF.Square, accum_out=ssq[:, j:j + 1])
                    rr = a_stat.tile([128, NH_T], F32, tag="rr")
                    nc.scalar.activation(rr[:], ssq[:], AF.Sqrt,
                                         scale=1.0 / HD, bias=eps_sb[:])
                    ri = a_stat.tile([128, NH_T], F32, tag="ri")
                    nc.vector.reciprocal(ri[:], rr[:])
                    rq = a_stat.tile([128, NH_T], F32, tag="rq")
                    nc.vector.tensor_mul(rq[:], ri[:], gains_sb[:])

                    # evict PSUM once: q/k heads to qkv_sb, V direct to v_sb
                    qkv_sb = a_sbuf.tile([128, NH_T * 128], BF16, tag="qkvsb")
                    nc.vector.tensor_copy(qkv_sb[:], qkv_ps[:, :NH_T * 128])
                    nc.vector.tensor_copy(
                        v_sb[:, st * NKV_L * HD:(st + 1) * NKV_L * HD],
                        qkv_ps[:, NH_T * 128:])

                    # batched rope in bf16 (2x DVE) with pre-tiled cos/sin
                    natq = a_sbuf.tile([128, NH_T * 128], BF16, tag="natq")
                    qv = qkv_sb[:].rearrange(
                        "p (h two s) -> p h two s", h=NH_T, two=2)
                    nv = natq[:].rearrange(
                        "p (h two s) -> p h two s", h=NH_T, two=2)
                    u1 = qv[:, :, 0, :]
                    u2 = qv[:, :, 1, :]
                    o1 = nv[:, :, 0, :]
                    o2 = nv[:, :, 1, :]
                    cob = cs_t[:, :NH_T * 64].rearrange(
                        "p (h s) -> p h s", h=NH_T)
                    sib = cs_t[:, NH_T * 64:].rearrange(
                        "p (h s) -> p h s", h=NH_T)
                    ta = a_stat.tile([128, NH_T * 64], BF16, tag="ta")
                    tb = a_stat.tile([128, NH_T * 64], BF16, tag="tb")
                    tav = ta[:].rearrange("p (h s) -> p h s", h=NH_T)
                    tbv = tb[:].rearrange("p (h s) -> p h s", h=NH_T)
                    nc.vector.tensor_mul(tav, u1, cob)
                    nc.vector.tensor_mul(tbv, u2, sib)
                    nc.vector.tensor_add(o1, tav, tbv)
                    nc.vector.tensor_mul(tav, u2, cob)
                    nc.vector.tensor_mul(tbv, u1, sib)
                    nc.vector.tensor_sub(o2, tav, tbv)
                    for j in range(NH_T):
                        nc.vector.tensor_scalar_mul(
                            natq[:, j * 128:(j + 1) * 128],
                            natq[:, j * 128:(j + 1) * 128], rq[:, j:j + 1])

                    # PE transposes into one psum tile, then 2 batched evicts
                    tp = t_psum.tile([128, NH_T * 128], BF16, tag="tp")
                    for j in range(NH_T):
                        nc.tensor.transpose(
                            tp[:, j * 128:(j + 1) * 128],
                            natq[:, j * 128:(j + 1) * 128], ident[:])
                    qdst = qt_sb[:].rearrange(
                        "p (h s) -> p h s", h=NH_L)[:, :, st * 128:(st + 1) * 128]
                    nc.vector.tensor_copy(
                        qdst,
                        tp[:, :NH_L * 128].rearrange("p (h s) -> p h s", h=NH_L))
                    kdst = kt_sb[:].rearrange(
                        "p (h s) -> p h s", h=NKV_L)[:, :, st * 128:(st + 1) * 128]
                    nc.vector.tensor_copy(
                        kdst,
                        tp[:, NH_L * 128:].rearrange("p (h s) -> p h s", h=NKV_L))

            # ---------------- stage B: attention per head
            with tc.tile_pool(name="b_sbuf", bufs=3) as b_sbuf, \
                 tc.tile_pool(name="b_acc", bufs=2) as b_acc, \
                 tc.tile_pool(name="b_rt", bufs=2) as b_rt, \
                 tc.tile_pool(name="b_rb", bufs=2) as b_rb, \
                 tc.tile_pool(name="s_psum", bufs=2, space="PSUM") as s_psum, \
                 tc.tile_pool(name="y_psum", bufs=2, space="PSUM") as y_psum, \
                 tc.tile_pool(name="d_psum", bufs=1, space="PSUM") as d_psum, \
                 tc.tile_pool(name="r_psum", bufs=1, space="PSUM") as r_psum:
                for j in range(NH_L):
                    jj = j // (NH_L // NKV_L)
                    q0 = j * SEQ  # column base of this head in qt_sb/yt_sb
                    for qt in range(N_QT):
                        nfull = 4 * qt
                        # units: list of (blocks, widths, q-offsets, tile-offsets)
                        # full pairs then two band pairs with causal trimming
                        units = []
                        for g in range(nfull // 2):
                            units.append([(2 * g, 512, 0, 0),
                                          (2 * g + 1, 512, 0, 512)])
                        units.append([(nfull + 0, 512, 0, 0),
                                      (nfull + 1, 384, 128, 512)])
                        units.append([(nfull + 2, 256, 256, 0),
                                      (nfull + 3, 128, 384, 256)])

                        acc = b_acc.tile([128, 512], F32, tag="acc")
                        y_ps = y_psum.tile([128, 512], F32, tag="y")
                        nblk = nfull + 4

                        def emit_y(unit, p_bf):
                            for kb, w, qo, to in unit:
                                nc.tensor.matmul(
                                    y_ps[:, qo:qo + w],
                                    v_sb[:, kb * NKV_L * HD + jj * HD:
                                         kb * NKV_L * HD + (jj + 1) * HD],
                                    p_bf[:, to:to + w],
                                    start=(kb == 0), stop=(kb == nblk - 1),
                                    skip_group_check=True)

                        pend = None
                        first_acc = True
                        for ui, unit in enumerate(units):
                            s_ps = s_psum.tile([128, 1024], F32, tag="s")
                            used = unit[-1][3] + unit[-1][1]
                            for kb, w, qo, to in unit:
                                nc.tensor.matmul(
                                    s_ps[:, to:to + w],
                                    kt_sb[:, jj * SEQ + kb * 128:
                                          jj * SEQ + (kb + 1) * 128],
                                    qt_sb[:, q0 + qt * 512 + qo:
                                          q0 + qt * 512 + qo + w],
                                    start=True, stop=True)
                            p_bf = b_sbuf.tile([128, 1024], BF16, tag="p")
                            nc.scalar.activation(
                                p_bf[:, :used], s_ps[:, :used], AF.Exp)
                            # zero the strict upper triangle of diagonal chunks
                            for kb, w, qo, to in unit:
                                d = kb - 4 * qt
                                if d >= 0:
                                    nc.gpsimd.affine_select(
                                        out=p_bf[:, to:to + 128],
                                        in_=p_bf[:, to:to + 128],
                                        compare_op=OP.is_ge,
                                        fill=0.0,
                                        base=0,
                                        pattern=[[1, 128]],
                                        channel_multiplier=-1)
                            # softmax denominator accumulation
                            if unit[0][1] == 512 and unit[1][1] == 512:
                                tmp = b_sbuf.tile([128, 512], BF16, tag="tmp")
                                nc.gpsimd.tensor_add(
                                    tmp[:], p_bf[:, :512], p_bf[:, 512:])
                                if first_acc:
                                    nc.vector.tensor_copy(acc[:], tmp[:])
                                    first_acc = False
                                else:
                                    nc.vector.tensor_add(acc[:], acc[:], tmp[:])
                            else:
                                for kb, w, qo, to in unit:
                                    if first_acc:
                                        nc.vector.tensor_copy(
                                            acc[:, qo:qo + w], p_bf[:, to:to + w])
                                        first_acc = False
                                    else:
                                        nc.vector.tensor_add(
                                            acc[:, qo:qo + w],
                                            acc[:, qo:qo + w], p_bf[:, to:to + w])
                            if pend is not None:
                                emit_y(*pend)
                            pend = (unit, p_bf)
                        emit_y(*pend)

                        # denominator: ones-matmul partition reduction,
                        # fast reciprocal, PE broadcast back to [128, 512]
                        dT = d_psum.tile([1, 512], F32, tag="d")
                        nc.tensor.matmul(dT[:], ones_col[:], acc[:],
                                         start=True, stop=True)
                        rt = b_rt.tile([1, 512], F32, tag="rt")
                        nc.vector.reciprocal_approx_fast(rt[:], dT[:])
                        rb_ps = r_psum.tile([128, 512], F32, tag="rb")
                        nc.tensor.matmul(rb_ps[:], ones_row[:], rt[:],
                                         start=True, stop=True)
                        rb_sb = b_rb.tile([128, 512], F32, tag="rbs")
                        nc.scalar.copy(rb_sb[:], rb_ps[:])
                        nc.vector.tensor_mul(
                            yt_sb[:, q0 + qt * 512:q0 + (qt + 1) * 512],
                            y_ps[:], rb_sb[:])

            # ---------------- stage C: output projection
            with tc.tile_pool(name="c_sbuf", bufs=2) as c_sbuf, \
                 tc.tile_pool(name="c_psum", bufs=2, space="PSUM") as c_psum:
                for ot in range(DIM // 128):
                    po_ps = c_psum.tile([128, SEQ], F32, tag="po")
                    for ic in range(NH_L):
                        for sc in range(SEQ // 512):
                            nc.tensor.matmul(
                                po_ps[:, sc * 512:(sc + 1) * 512],
                                wp_sb[ic][:, ot * 128:(ot + 1) * 128],
                                yt_sb[:, ic * SEQ + sc * 512: ic * SEQ + (sc + 1) * 512],
                                start=(ic == 0), stop=(ic == NH_L - 1))
                    o_sb = c_sbuf.tile([128, SEQ], F32, tag="osb")
                    nc.vector.tensor_copy(o_sb[:, :1024], po_ps[:, :1024])
                    nc.scalar.copy(o_sb[:, 1024:], po_ps[:, 1024:])
                    nc.sync.dma_start(out[ot * 128:(ot + 1) * 128, :], o_sb[:])

    nc.compile()
    return nc


def _get_nc():
    global _CACHED_NC
    if _CACHED_NC is None:
        _CACHED_NC = _build_nc()
    return _CACHED_NC


def _make_csrep():
    half = HD // 2
    inv_freq = 1.0 / (10000.0 ** (np.arange(0, HD, 2, dtype=np.float32) / HD))
    t = np.arange(SEQ, dtype=np.float32)
    freqs = np.outer(t, inv_freq)  # [S, 64]
    cos = np.tile(np.cos(freqs), (1, NH_T))
    sin = np.tile(np.sin(freqs), (1, NH_T))
    return np.ascontiguousarray(
        np.concatenate([cos, sin], axis=1)).astype(ml_dtypes.bfloat16)


def _prep_inputs(x, Wq, Wk, Wv, Wproj, q_gain):
    bf = ml_dtypes.bfloat16
    csrep = _make_csrep()
    xT = [np.ascontiguousarray(x[b].T).astype(bf) for b in range(BATCH)]
    wt_h, wp_h, g_h = [], [], []
    for h in range(2):
        w = np.concatenate([
            Wq[1024 * h:1024 * (h + 1)],
            Wk[256 * h:256 * (h + 1)],
            Wv[256 * h:256 * (h + 1)]], axis=0)
        wt_h.append(np.ascontiguousarray(w.T).astype(bf))
        wp_h.append(np.ascontiguousarray(
            Wproj[:, 1024 * h:1024 * (h + 1)].T).astype(bf))
        g = np.concatenate([
            q_gain[8 * h:8 * (h + 1)] / np.sqrt(HD),
            np.ones(NKV_L, np.float32)]).astype(np.float32)
        g_h.append(np.ascontiguousarray(
            np.broadcast_to(g[None, :], (128, NH_T))))

    in_maps = []
    for c in range(N_CORES):
        b, h = c // 2, c % 2
        in_maps.append({
            "xT": xT[b], "wt": wt_h[h], "wpT": wp_h[h], "csrep": csrep,
            "gains": g_h[h],
        })
    return in_maps


def kernel(x, Wq, Wk, Wv, Wproj, q_gain):
    x = np.asarray(x, dtype=np.float32)
    Wq = np.asarray(Wq, dtype=np.float32)
    Wk = np.asarray(Wk, dtype=np.float32)
    Wv = np.asarray(Wv, dtype=np.float32)
    Wproj = np.asarray(Wproj, dtype=np.float32)
    q_gain = np.asarray(q_gain, dtype=np.float32)

    in_maps = _prep_inputs(x, Wq, Wk, Wv, Wproj, q_gain)
    nc = _get_nc()
    res = run_bass_kernel_spmd(nc, in_maps, list(range(N_CORES))).results

    out = np.empty((BATCH, SEQ, DIM), dtype=np.float32)
    for b in range(BATCH):
        out[b] = (res[2 * b]["out"] + res[2 * b + 1]["out"]).T
    return out
